# revision 13
# baseline (speedup 1.0000x reference)
"""CombinedDynamicMarginLoss on 8 trn2 NeuronCores.

Strategy: data-parallel over the batch dim N=1024 -> 128 rows per core
(one full SBUF partition tile), each core sees all C=93431 classes so
every per-row reduction is core-local (no collectives).

Mixed precision: the 2e-2 relative-error gate admits bf16 end-to-end
(worst-case ~4e-3), so the bulk [N, C] stream is bf16 in both
directions — the host casts logits to bf16 once, the device streams
bf16 in and bf16 out. That halves HBM traffic on both sides vs f32
(memory-bound kernel -> ~2x).

Device per core (streaming, single pass over the 23.9MB shard):
  - out = bf16(64 * x)        (ACT engine; exact given bf16 input —
                               x64 shifts the exponent only)
  - rowmax_t = max_j min(x_ij, 0.4f)  (ONE DVE tensor_scalar per tile:
                               op0=min clamps, op1+accum_out reduces)
    min(x, 0.4f) is the interclass filter in disguise: kept bf16
    values (<= 0.3984375) pass through exactly; filtered values
    (>= 0.400390625) clamp to 0.4f, which upper-bounds the true
    filtered max and is within ~1e-4 of it for any realistically
    dense row (the flagged rows below get an exact f32 recompute).
Host glue (1024 rows, negligible):
  - bf16 -> f32 upcast of the bulk output, cos_y gather from the
    original f32 logits, margin math, scatter of final_phi*64.
  - Exact f32 recompute of max_other for the few rows where the
    device's approximation could matter: |phi*64| small (error
    amplification), M suspiciously low (sparse row), or the label
    column itself near the max.
"""

import os
import time

import ml_dtypes
import numpy as np

os.environ.setdefault("NEURON_RT_RESET_CORES", "1")

import concourse.bacc as bacc
import concourse.mybir as mybir
import concourse.tile as tile
from concourse.bass_utils import run_bass_kernel_spmd

N, C = 1024, 93431
NCORES = 8
R = N // NCORES  # 128 rows per core

S = 64.0
M1 = 1.0
M2 = 0.5
M3 = 0.0
ALPHA = 0.1
THRESH = 0.4
NEG_BIG = -1.0e9

BF16 = np.dtype(ml_dtypes.bfloat16)

T = 8192                      # column tile buffer width
# Variable tile widths: a small first tile starts the store stream early,
# a small last tile minimizes the compute-drain after the final load.
WIDTHS = [512] + [8192] * 11 + [2294] + [513]
assert sum(WIDTHS) == C and max(WIDTHS) == T
NT = len(WIDTHS)              # 14

_CACHE: dict = {}
LAST_RESULT = None            # BassKernelResults of the last run (for test.py)
RUN_KWARGS: dict = {}         # test.py can set {"trace": True}


def _build():
    f32 = mybir.dt.float32
    bf16 = mybir.dt.bfloat16
    # Bacc (not raw Bass): its compile pass splits multi-wait sync onto
    # separate event-semaphore instructions — DMACopy only encodes 1 wait.
    nc = bacc.Bacc(None, enable_partition_id=False)
    x = nc.declare_dram_parameter("x", [R, C], bf16, isOutput=False)
    y = nc.declare_dram_parameter("y", [R, C], bf16, isOutput=True)
    mx = nc.declare_dram_parameter("mx", [R, NT], f32, isOutput=True)

    # Loads on the sync HWDGE ring, stores on the scalar engine's HWDGE
    # ring (same-engine ordering after the mul that produced the data).
    with tile.TileContext(nc) as tc:
        with (
            tc.tile_pool(name="xin", bufs=5) as xpool,
            tc.tile_pool(name="yout", bufs=4) as ypool,
            tc.tile_pool(name="gtmp", bufs=2) as gpool,
            tc.tile_pool(name="stat", bufs=1) as statpool,
        ):
            maxbuf = statpool.tile([R, NT], f32)
            col = 0
            tail_stores = []  # emitted after all loads: drain on the idle sync ring
            for t, w in enumerate(WIDTHS):
                xt = xpool.tile([R, T], bf16, tag="xt")
                nc.sync.dma_start(out=xt[:, :w], in_=x[:, col : col + w])

                yt = ypool.tile([R, T], bf16, tag="yt")
                nc.scalar.mul(yt[:, :w], xt[:, :w], S)
                if t >= NT - 3:
                    tail_stores.append((yt, col, w))
                else:
                    nc.scalar.dma_start(out=y[:, col : col + w], in_=yt[:, :w])

                # rowmax_t = max_j min(x, 0.4): clamp-filter + max-reduce
                # in one DVE op (accum_out repurposes op1 as the reduce op)
                g = gpool.tile([R, T], bf16, tag="g")
                nc.vector.tensor_scalar(
                    out=g[:, :w],
                    in0=xt[:, :w],
                    scalar1=THRESH,
                    scalar2=None,
                    op0=mybir.AluOpType.min,
                    op1=mybir.AluOpType.max,
                    accum_out=maxbuf[:, t : t + 1],
                )
                col += w

            # tail stores ride the sync ring (its loads are done by then),
            # halving the store-drain at the end of the kernel
            for yt, c0, w in tail_stores:
                nc.sync.dma_start(out=y[:, c0 : c0 + w], in_=yt[:, :w])

            # ship the per-tile maxima; the final NT-column max runs on host
            nc.scalar.dma_start(out=mx[:], in_=maxbuf[:])
    # run_bass_via_pjrt serializes the module at jit-lowering time without
    # finalizing; Bacc's register allocation happens in finalize().
    nc.finalize()
    return nc


def _get_nc():
    if "nc" not in _CACHE:
        _CACHE["nc"] = _build()
    return _CACHE["nc"]


def _exact_max_other(logits_row, label):
    """f32-exact max over non-label classes after interclass filtering.

    The threshold compare stays in f32 (matching the reference's weak-typed
    jnp compare) — a Python-float 0.4 would flip values equal to f32(0.4).
    """
    g = np.where(logits_row <= np.float32(THRESH), logits_row, 0.0).astype(np.float32)
    g[label] = NEG_BIG
    return g.max()


def kernel(logits, labels):
    global LAST_RESULT
    logits = np.ascontiguousarray(np.asarray(logits, dtype=np.float32))
    labels = np.asarray(labels).astype(np.int64)
    assert logits.shape == (N, C)

    xb = logits.astype(BF16)  # round-to-nearest-even, one rounding

    nc = _get_nc()
    in_maps = [{"x": xb[k * R : (k + 1) * R]} for k in range(NCORES)]
    # transient NRT_EXEC_UNIT_UNRECOVERABLE wedges happen occasionally on
    # this fabric; a retry after a short pause recovers the device
    last_exc = None
    for attempt in range(3):
        try:
            res = run_bass_kernel_spmd(nc, in_maps, list(range(NCORES)), **RUN_KWARGS)
            break
        except Exception as e:  # noqa: BLE001 — device-level flake, retry
            last_exc = e
            time.sleep(3.0 * (attempt + 1))
    else:
        raise last_exc
    LAST_RESULT = res

    # upcast the bf16 device output to f32 (bulk of the result)
    out = np.empty((N, C), dtype=np.float32)
    for k in range(NCORES):
        out[k * R : (k + 1) * R] = res.results[k]["y"]
    M = np.concatenate([res.results[k]["mx"] for k in range(NCORES)], axis=0).max(axis=1)

    # ---- host glue: per-row scalars (N=1024) ----
    valid = labels != -1
    lab = np.where(valid, labels, 0)
    rows = np.arange(N)
    cos_y = logits[rows, lab]                                   # exact f32

    max_other = M.copy()
    # sparse-row anomaly: M should be ~0.4 (clamped filtered columns);
    # anything lower means the label column could have set the max or the
    # clamp approximation is off — recompute exactly
    need_exact = (M < 0.399) & valid

    h = (np.float32(1.0) - (cos_y - max_other)).astype(np.float32)
    m_i = (np.float32(M2) + np.float32(ALPHA) * h).astype(np.float32)
    theta = np.arccos(np.clip(cos_y, -1.0, 1.0)).astype(np.float32)
    phi = (np.cos(np.float32(M1) * theta + m_i) - np.float32(M3)).astype(np.float32)

    # rows where a small max_other error could breach the 2e-2 relative
    # gate on the scattered value (|phi|*64 small)
    need_exact |= (np.abs(phi) * np.float32(S) < 2.0) & valid

    for i in np.nonzero(need_exact)[0]:
        max_other[i] = _exact_max_other(logits[i], lab[i])
    h = (np.float32(1.0) - (cos_y - max_other)).astype(np.float32)
    m_i = (np.float32(M2) + np.float32(ALPHA) * h).astype(np.float32)
    phi = (np.cos(np.float32(M1) * theta + m_i) - np.float32(M3)).astype(np.float32)
    final_phi = np.where(phi < cos_y, phi, cos_y).astype(np.float32)

    out[rows[valid], lab[valid]] = final_phi[valid] * np.float32(S)
    return out
